# revision 1
# baseline (speedup 1.0000x reference)
"""Multi-head attention (B=2, S=2048, D=1024, H=16) on 8 TRN2 NeuronCores.

Sharding: (batch, head-group) SPMD. Core c handles batch b = c//4 and local
heads [4*(c%4), 4*(c%4)+4). Each core computes its 4 heads' attention plus the
partial o-projection (row-parallel over the head dimension); the host sums the
4 partial outputs per batch (the all-reduce of row-parallel o_proj) and adds
b_o.

Device dataflow per core (all matmuls fp32r except P@V in bf16):
  phase 1: QT = (wq/8) @ x.T, KT = wk @ x.T   (transposed layouts [ch, seq])
           V  = x @ wv.T                       ([seq, ch] chunks, bf16, with a
                                                ones column per head for the
                                                softmax denominator)
  phase 2: per q-block (512) and k-tile (128):
           T = S.T block   = KT_h.T @ QT_h     (PSUM, K=64 row-packed pairs)
           E = exp(T)                          (ACT, PSUM->SBUF bf16)
           P.T = E * maskT                     (DVE, bf16 2x mode)
           Cq += [V_h|1].T @ P.T               (PSUM accumulate, row 64 = den)
           then rec = exp(-ln(den)), partition-broadcast, Cn = Cq * rec,
           y.T += woT_h.T @ Cn                 (fp32r), DMA out.
"""
import os
import sys

if "/opt/trn_rl_repo" not in sys.path:
    sys.path.insert(0, "/opt/trn_rl_repo")
os.environ.setdefault("JAX_PLATFORMS", "axon,cpu")

from contextlib import ExitStack

import ml_dtypes
import numpy as np

import concourse.bass as bass
import concourse.tile as tile
from concourse import bacc, library_config, mybir
from concourse.bass_utils import run_bass_kernel_spmd

F32 = mybir.dt.float32
F32R = mybir.dt.float32r
BF16 = mybir.dt.bfloat16
EXP = mybir.ActivationFunctionType.Exp
LN = mybir.ActivationFunctionType.Ln

B, S, D = 2, 2048, 1024
H, HD = 16, 64
HL = 4            # local heads per core
CH = HL * HD      # 256 local channels
N_CORES = 8
KC = D // 128     # 8 contraction chunks for the projections
NQB = S // 512    # 4 q blocks
NKT = S // 128    # 16 k tiles

_CACHE = {}


def _build_nc(debug=False):
    nc = bacc.Bacc("TRN2", target_bir_lowering=False)
    xT_d = nc.declare_dram_parameter("xT", [D, S], F32R, isOutput=False)
    mk_d = nc.declare_dram_parameter("maskT", [S, S], BF16, isOutput=False)
    wqT_d = nc.declare_dram_parameter("wqT", [D, CH], F32R, isOutput=False)
    wkT_d = nc.declare_dram_parameter("wkT", [D, CH], F32R, isOutput=False)
    wvT_d = nc.declare_dram_parameter("wvT", [D, CH], F32R, isOutput=False)
    woT_d = nc.declare_dram_parameter("woT", [CH, D], BF16, isOutput=False)
    yT_d = nc.declare_dram_parameter("yT", [D, S], F32, isOutput=True)
    dbg = {}
    if debug:
        dbg["qt0"] = nc.declare_dram_parameter("d_qt0", [128, S], F32, isOutput=True)
        dbg["kt0"] = nc.declare_dram_parameter("d_kt0", [128, S], F32, isOutput=True)
        dbg["v0"] = nc.declare_dram_parameter("d_v0", [128, HL * 65], BF16, isOutput=True)
        dbg["tq"] = nc.declare_dram_parameter("d_tq", [128, 1024], F32, isOutput=True)
        dbg["ex"] = nc.declare_dram_parameter("d_ex", [128, 1024], BF16, isOutput=True)
        dbg["pt"] = nc.declare_dram_parameter("d_pt", [128, 1024], BF16, isOutput=True)
        dbg["cq"] = nc.declare_dram_parameter("d_cq", [65, HL * 512], F32, isOutput=True)
        dbg["rec"] = nc.declare_dram_parameter("d_rec", [1, HL * 512], F32, isOutput=True)
        dbg["recb"] = nc.declare_dram_parameter("d_recb", [64, HL * 512], F32, isOutput=True)
        dbg["cn"] = nc.declare_dram_parameter("d_cn", [64, HL * 512], F32, isOutput=True)

    with tile.TileContext(nc) as tc, ExitStack() as ctx:
        nc.gpsimd.load_library(library_config.attn)
        const = ctx.enter_context(tc.tile_pool(name="const", bufs=1))
        psum = ctx.enter_context(tc.tile_pool(name="psum", bufs=1, space="PSUM"))

        # ---- resident tensors ----
        mk = [const.tile([128, S], BF16, name=f"mk{kt}") for kt in range(NKT)]
        wo = []
        for h in range(HL):
            t = const.tile([64, D], BF16, name=f"wo{h}")
            nc.sync.dma_start(t[:], woT_d[h * 64:(h + 1) * 64, :])
            wo.append(t)
        # persistent QT/KT ([2 heads * 64 d, seq] pair tiles) and V chunks
        qt = [const.tile([128, S], BF16, name=f"qt{i}") for i in range(2)]
        kt_sb = [const.tile([128, S], BF16, name=f"kt{i}") for i in range(2)]
        v_sb = [const.tile([128, HL * 65], BF16, name=f"v{i}") for i in range(NKT)]
        for st in range(NKT):
            # ones column per head (softmax denominator trick)
            nc.gpsimd.memset(
                v_sb[st].rearrange("p (h c) -> p h c", h=HL)[:, :, 64:65], 1.0
            )

        # ---- phase 1: projections (own pool, closed before phase 2) ----
        with tc.tile_pool(name="p1", bufs=1) as p1:
            wq_sb = [p1.tile([128, CH], F32R, name=f"wq{k}") for k in range(KC)]
            wk_sb = [p1.tile([128, CH], F32R, name=f"wk{k}") for k in range(KC)]
            wv_sb = [p1.tile([128, CH], F32R, name=f"wv{k}") for k in range(KC)]
            for k in range(KC):
                nc.sync.dma_start(wq_sb[k][:], wqT_d[k * 128:(k + 1) * 128, :])
                nc.sync.dma_start(wk_sb[k][:], wkT_d[k * 128:(k + 1) * 128, :])
                nc.sync.dma_start(wv_sb[k][:], wvT_d[k * 128:(k + 1) * 128, :])

            for qh in range(4):  # seq quarters of 512
                xt = []
                for k in range(KC):
                    t = p1.tile([128, 512], F32R, name=f"xt{k}", bufs=1)
                    nc.sync.dma_start(
                        t[:], xT_d[k * 128:(k + 1) * 128, qh * 512:(qh + 1) * 512]
                    )
                    xt.append(t)

                # interleave Q/K m-tiles with V seq-tiles for PE overlap
                for j, (wsb, dst, mt) in enumerate(
                    [(wq_sb, qt, 0), (wq_sb, qt, 1), (wk_sb, kt_sb, 0), (wk_sb, kt_sb, 1)]
                ):
                    ps = psum.tile([128, 512], F32, name="psa", tag="psa", bufs=2)
                    for k in range(KC):
                        nc.tensor.matmul(
                            ps[:],
                            wsb[k][:, mt * 128:(mt + 1) * 128],
                            xt[k][:],
                            start=(k == 0), stop=(k == KC - 1),
                        )
                    nc.scalar.copy(dst[mt][:, qh * 512:(qh + 1) * 512], ps[:])
                    if j % 2 == 0:  # 2 V seq-tiles after every other QK job
                        for st_l in range(2):
                            sl = j + st_l
                            st = qh * 4 + sl
                            vp = psum.tile([128, CH], F32, name="psb", tag="psb", bufs=1)
                            for k in range(KC):
                                nc.tensor.matmul(
                                    vp[:],
                                    xt[k][:, sl * 128:(sl + 1) * 128],
                                    wv_sb[k][:],
                                    start=(k == 0), stop=(k == KC - 1),
                                )
                            nc.vector.tensor_copy(
                                v_sb[st].rearrange("p (h c) -> p h c", h=HL)[:, :, 0:64],
                                vp.rearrange("p (h c) -> p h c", h=HL),
                            )

        # mask loads issued after phase-1 inputs: first consumer is phase 2
        for kt in range(NKT):
            nc.sync.dma_start(mk[kt][:], mk_d[kt * 128:(kt + 1) * 128, :])

        if debug:
            nc.gpsimd.dma_start(dbg["qt0"][:], qt[0][:])
            nc.gpsimd.dma_start(dbg["kt0"][:], kt_sb[0][:])
            nc.sync.dma_start(dbg["v0"][:], v_sb[0][:])

        # ---- phase 2: attention + o_proj ----
        with tc.tile_pool(name="work", bufs=1) as work:
            for qb in range(NQB):
                cq = psum.tile([128, HL * 512], F32, name="psb", tag="psb", bufs=1)
                for ktile in range(NKT):
                    for pair in range(2):
                        tq = psum.tile([128, 1024], F32, name="psa", tag="psa", bufs=2)
                        for hh in range(2):
                            nc.tensor.matmul(
                                tq[:, hh * 512:(hh + 1) * 512],
                                kt_sb[pair][hh * 64:(hh + 1) * 64,
                                            ktile * 128:(ktile + 1) * 128],
                                qt[pair][hh * 64:(hh + 1) * 64,
                                         qb * 512:(qb + 1) * 512],
                                start=True, stop=True,
                            )
                        if debug and qb == 0 and ktile == 0 and pair == 0:
                            tqc = work.tile([128, 1024], F32, name="tqc", tag="ysb2", bufs=1)
                            nc.vector.tensor_copy(tqc[:], tq[:])
                            nc.sync.dma_start(dbg["tq"][:], tqc[:])
                        ex = work.tile([128, 1024], BF16, name="expq", tag="expq", bufs=4)
                        nc.scalar.activation(ex[:], tq[:], EXP)
                        if debug and qb == 0 and ktile == 0 and pair == 0:
                            nc.sync.dma_start(dbg["ex"][:], ex[:])
                        pt = work.tile([128, 1024], BF16, name="pt", tag="pt", bufs=6)
                        for hh in range(2):
                            nc.vector.tensor_mul(
                                pt[:, hh * 512:(hh + 1) * 512],
                                ex[:, hh * 512:(hh + 1) * 512],
                                mk[ktile][:, qb * 512:(qb + 1) * 512],
                            )
                        if debug and qb == 0 and ktile == 0 and pair == 0:
                            nc.sync.dma_start(dbg["pt"][:], pt[:])
                        for hh in range(2):
                            h = pair * 2 + hh
                            nc.tensor.matmul(
                                cq[0:65, h * 512:(h + 1) * 512],
                                v_sb[ktile][:, h * 65:h * 65 + 65],
                                pt[:, hh * 512:(hh + 1) * 512],
                                start=(ktile == 0), stop=(ktile == NKT - 1),
                            )
                # softmax denominator: rec = exp(-ln(den)) = 1/den
                if debug and qb == 0:
                    cqc = work.tile([65, HL * 512], F32, name="cqc", tag="ysb2", bufs=1)
                    nc.vector.tensor_copy(cqc[:], cq[0:65, :])
                    nc.sync.dma_start(dbg["cq"][:], cqc[:])
                nc.scalar.activation(cq[64:65, :], cq[64:65, :], LN)
                rec = work.tile([65, HL * 512], F32, name="rec", tag="cn", bufs=2)
                nc.scalar.activation(rec[64:65, :], cq[64:65, :], EXP, scale=-1.0)
                # hop the row to partition 0 via DMA: partition_broadcast's
                # ucode broadcasts the tile's partition 0 on hardware
                rec0 = work.tile([1, HL * 512], F32, name="rec0", tag="ysb2", bufs=1)
                nc.sync.dma_start(rec0[:], rec[64:65, :])
                rb = work.tile([64, HL * 512], F32, name="recb", tag="recb", bufs=1)
                nc.gpsimd.partition_broadcast(rb[:], rec0[:])
                cn = work.tile([64, HL * 512], BF16, name="cn", tag="cn", bufs=2)
                nc.vector.tensor_mul(cn[:], cq[0:64, :], rb[:])
                if debug and qb == 0:
                    nc.sync.dma_start(dbg["rec"][:], rec[64:65, :])
                    nc.sync.dma_start(dbg["recb"][:], rb[:])
                    nc.gpsimd.dma_start(dbg["cn"][:], cn[:])

                for g in range(2):  # two groups of 4 output tiles
                    op = psum.tile([128, 2048], F32, name="psb", tag="psb", bufs=1)
                    for ot_l in range(4):
                        ot = g * 4 + ot_l
                        for h in range(HL):
                            nc.tensor.matmul(
                                op[:, ot_l * 512:(ot_l + 1) * 512],
                                wo[h][:, ot * 128:(ot + 1) * 128],
                                cn[:, h * 512:(h + 1) * 512],
                                start=(h == 0), stop=(h == HL - 1),
                            )
                    ysb = work.tile([128, 2048], F32, name="ysb", tag="ysb", bufs=2)
                    nc.vector.tensor_copy(ysb[:], op[:])
                    nc.sync.dma_start(
                        yT_d[g * 512:(g + 1) * 512,
                             qb * 512:(qb + 1) * 512].rearrange("(o r) c -> r o c", o=4),
                        ysb.rearrange("r (o c) -> r o c", o=4),
                    )
    nc.compile()
    return nc


def _get_nc(debug=False):
    key = ("nc", debug)
    if key not in _CACHE:
        _CACHE[key] = _build_nc(debug)
    return _CACHE[key]


def kernel(x, mask, w_qkv, b_qkv, w_o, b_o):
    x = np.asarray(x, dtype=np.float32)
    mask = np.asarray(mask)
    w_qkv = np.asarray(w_qkv, dtype=np.float32)
    b_qkv = np.asarray(b_qkv, dtype=np.float32)
    w_o = np.asarray(w_o, dtype=np.float32)
    b_o = np.asarray(b_o, dtype=np.float32)
    assert not b_qkv.any(), "kernel specialized for zero qkv bias"

    scale = np.float32(1.0 / np.sqrt(HD))
    maskT = np.ascontiguousarray(mask.reshape(S, S).T).astype(ml_dtypes.bfloat16)

    w3 = w_qkv.reshape(H, 3, HD, D)  # [head, (q,k,v), hd, D]
    in_maps = []
    for c in range(N_CORES):
        b = c // 4
        h0 = (c % 4) * HL
        heads = list(range(h0, h0 + HL))
        wq = w3[heads, 0].reshape(CH, D) * scale
        wk = w3[heads, 1].reshape(CH, D)
        wv = w3[heads, 2].reshape(CH, D)
        wo_cols = np.concatenate([w_o[:, h * HD:(h + 1) * HD] for h in heads], axis=1)
        in_maps.append({
            "xT": np.ascontiguousarray(x[b].T),
            "maskT": maskT,
            "wqT": np.ascontiguousarray(wq.T),
            "wkT": np.ascontiguousarray(wk.T),
            "wvT": np.ascontiguousarray(wv.T),
            "woT": np.ascontiguousarray(wo_cols.T).astype(ml_dtypes.bfloat16),
        })

    nc = _get_nc()
    trace = bool(int(os.environ.get("MHA_TRACE", "0")))
    res = run_bass_kernel_spmd(nc, in_maps, core_ids=list(range(N_CORES)),
                               trace=trace)
    _CACHE["last_results"] = res

    y = np.zeros((B, S, D), dtype=np.float32)
    for c in range(N_CORES):
        y[c // 4] += res.results[c]["yT"].T
    y += b_o
    return y



# revision 5
# speedup vs baseline: 1.2286x; 1.2286x over previous
"""Multi-head attention (B=2, S=2048, D=1024, H=16) on 8 TRN2 NeuronCores.

Sharding: (batch, head-group) SPMD. Core c handles batch b = c//4 and local
heads [4*(c%4), 4*(c%4)+4). Each core computes its 4 heads' attention plus the
partial o-projection (row-parallel over the head dimension); the host sums the
4 partial outputs per batch and adds b_o.

v2: software-pipelined single-pass schedule tuned from the v1 trace:
  - phase-1 projection quarters interleaved into qb0's attention stream
  - tensor queue ordered S(i) ... PV(i-2) so the PE never micro-idles
    (v1 lost ~45% of time to HAM clock-throttle from PE gaps)
  - ACT runs Exp only (denominator reciprocal on DVE; no LN table thrash)
  - qb boundary: cnr copy on ACT frees the PSUM accumulator early;
    1/den broadcast via DMA hop + gpsimd off the critical engines;
    o_proj chunks interleaved with the next qb's S stream
  - bf16 inputs (xT, weights, mask) to halve input DMA
"""
import os
import sys

if "/opt/trn_rl_repo" not in sys.path:
    sys.path.insert(0, "/opt/trn_rl_repo")
os.environ.setdefault("JAX_PLATFORMS", "axon,cpu")

from contextlib import ExitStack

import ml_dtypes
import numpy as np

import concourse.bass as bass
import concourse.tile as tile
from concourse import bacc, library_config, mybir
from concourse.bass_utils import run_bass_kernel_spmd

F32 = mybir.dt.float32
BF16 = mybir.dt.bfloat16
EXP = mybir.ActivationFunctionType.Exp

B, S, D = 2, 2048, 1024
H, HD = 16, 64
HL = 4            # local heads per core
CH = HL * HD      # 256 local channels
N_CORES = 8
KC = D // 128     # 8 contraction chunks for the projections
NKT = S // 128    # 16 k tiles
NIT = NKT * 2     # 32 (ktile, pair) iterations per q block
LAG = 2           # PV trails S by this many iterations in the tensor queue

_CACHE = {}


def _build_nc():
    nc = bacc.Bacc("TRN2", target_bir_lowering=False)
    xT_d = nc.declare_dram_parameter("xT", [D, S], BF16, isOutput=False)
    mk_d = nc.declare_dram_parameter("maskT", [S, S], BF16, isOutput=False)
    wqT_d = nc.declare_dram_parameter("wqT", [D, CH], BF16, isOutput=False)
    wkT_d = nc.declare_dram_parameter("wkT", [D, CH], BF16, isOutput=False)
    wvT_d = nc.declare_dram_parameter("wvT", [D, CH], BF16, isOutput=False)
    woT_d = nc.declare_dram_parameter("woT", [CH, D], BF16, isOutput=False)
    yT_d = nc.declare_dram_parameter("yT", [D, S], F32, isOutput=True)
    # scratch for the per-qb denominator reshape dance (slot 0: den row,
    # slot 1: reciprocal); declared as an ignored output
    scr_d = nc.declare_dram_parameter("scr", [4, 2, 2048], F32, isOutput=True)

    with tile.TileContext(nc) as tc, ExitStack() as ctx:
        nc.gpsimd.load_library(library_config.attn)
        const = ctx.enter_context(tc.tile_pool(name="const", bufs=1))
        work = ctx.enter_context(tc.tile_pool(name="work", bufs=1))
        psum = ctx.enter_context(tc.tile_pool(name="psum", bufs=1, space="PSUM"))

        # ---- resident tensors ----
        mk = [const.tile([128, S], BF16, name=f"mk{t}") for t in range(NKT)]
        wo = []
        for h in range(HL):
            t = const.tile([64, D], BF16, name=f"wo{h}")
            nc.sync.dma_start(t[:], woT_d[h * 64:(h + 1) * 64, :])
            wo.append(t)
        qt = [const.tile([128, S], BF16, name=f"qt{i}") for i in range(2)]
        kt_sb = [const.tile([128, S], BF16, name=f"kt{i}") for i in range(2)]
        v_sb = [const.tile([128, HL * 65], BF16, name=f"v{i}") for i in range(NKT)]
        for st in range(NKT):
            nc.gpsimd.memset(
                v_sb[st].rearrange("p (h c) -> p h c", h=HL)[:, :, 64:65], 1.0
            )
        wq_sb = [const.tile([128, CH], BF16, name=f"wq{k}") for k in range(KC)]
        wk_sb = [const.tile([128, CH], BF16, name=f"wk{k}") for k in range(KC)]
        wv_sb = [const.tile([128, CH], BF16, name=f"wv{k}") for k in range(KC)]
        for k in range(KC):
            nc.sync.dma_start(wq_sb[k][:], wqT_d[k * 128:(k + 1) * 128, :])
            nc.sync.dma_start(wk_sb[k][:], wkT_d[k * 128:(k + 1) * 128, :])
            nc.sync.dma_start(wv_sb[k][:], wvT_d[k * 128:(k + 1) * 128, :])

        # ---- phase-1 quarter: loads + KT/V/QT projections for seq quarter qh
        def quarter(qh):
            xts = []
            for k in range(KC):
                t = work.tile([128, 512], BF16, name=f"xt{k}", tag=f"xt{k}", bufs=2)
                nc.sync.dma_start(
                    t[:], xT_d[k * 128:(k + 1) * 128, qh * 512:(qh + 1) * 512]
                )
                xts.append(t)
            for j in range(4):
                kt = qh * 4 + j
                nc.sync.dma_start(mk[kt][:], mk_d[kt * 128:(kt + 1) * 128, :])
            # KT m-tiles (pair p = heads 2p,2p+1 = CH rows 128p:128p+128)
            for wsb, dsts in ((wk_sb, kt_sb), (wq_sb, qt)):
                ps = psum.tile([128, 1024], F32, name="p1qk", tag="psa", bufs=2)
                for mt in range(2):
                    for k in range(KC):
                        nc.tensor.matmul(
                            ps[:, mt * 512:(mt + 1) * 512],
                            wsb[k][:, mt * 128:(mt + 1) * 128],
                            xts[k][:],
                            start=(k == 0), stop=(k == KC - 1),
                        )
                for mt in range(2):
                    nc.scalar.copy(
                        dsts[mt][:, qh * 512:(qh + 1) * 512],
                        ps[:, mt * 512:(mt + 1) * 512],
                    )
                if wsb is wk_sb:
                    vp = psum.tile([128, 1024], F32, name="p1v", tag="psa", bufs=2)
                    for sl in range(4):
                        for k in range(KC):
                            nc.tensor.matmul(
                                vp[:, sl * 256:(sl + 1) * 256],
                                xts[k][:, sl * 128:(sl + 1) * 128],
                                wv_sb[k][:],
                                start=(k == 0), stop=(k == KC - 1),
                            )
                    for sl in range(4):
                        st = qh * 4 + sl
                        nc.scalar.copy(
                            v_sb[st].rearrange("p (h c) -> p h c", h=HL)[:, :, 0:64],
                            vp[:, sl * 256:(sl + 1) * 256].rearrange(
                                "p (h c) -> p h c", h=HL
                            ),
                        )

        # ---- qb boundary: free cq fast, then normalize off-critical-path
        bnd = {}

        def boundary_head(qb, cq):
            # one ACT copy of ctx+den rows frees cq; the den row then takes a
            # DRAM reshape round-trip so the reciprocal runs on 128 lanes
            cnr = work.tile([65, 2048], F32, name="cnr", tag="cnr", bufs=2)
            nc.scalar.copy(cnr[:], cq[0:65, :])
            nc.sync.dma_start(
                scr_d[qb, 0, :].rearrange("(a c) -> a c", a=1), cnr[64:65, :]
            )
            r128 = work.tile([128, 16], F32, name="r128", tag="r128", bufs=1)
            nc.sync.dma_start(
                r128[:], scr_d[qb, 0, :].rearrange("(p c) -> p c", p=128)
            )
            nc.vector.reciprocal(r128[:], r128[:])
            nc.sync.dma_start(
                scr_d[qb, 1, :].rearrange("(p c) -> p c", p=128), r128[:]
            )
            rb = work.tile([64, 2048], F32, name="rb", tag="rb", bufs=1)
            scr1 = scr_d[qb, 1, :]
            nc.sync.dma_start(
                rb[:], bass.AP(scr1.tensor, scr1.offset, [(0, 64), (1, 2048)])
            )
            bnd["cnr"], bnd["rb"] = cnr, rb

        def boundary_cn(qb):
            cn = work.tile([64, 2048], BF16, name="cn", tag="cn", bufs=2)
            nc.vector.tensor_mul(cn[:], bnd["cnr"][0:64, :], bnd["rb"][:])
            bnd["cn"] = cn

        def op_chunk(qb, g):
            cn = bnd["cn"]
            op = psum.tile([128, 1024], F32, name="op", tag="psa", bufs=2)
            for j in range(2):
                ot = g * 2 + j
                for h in range(HL):
                    nc.tensor.matmul(
                        op[:, j * 512:(j + 1) * 512],
                        wo[h][:, ot * 128:(ot + 1) * 128],
                        cn[:, h * 512:(h + 1) * 512],
                        start=(h == 0), stop=(h == HL - 1),
                    )
            ysb = work.tile([128, 1024], F32, name="ysb", tag="ysb", bufs=2)
            nc.vector.tensor_copy(ysb[:], op[:])
            nc.sync.dma_start(
                yT_d[g * 256:(g + 1) * 256,
                     qb * 512:(qb + 1) * 512].rearrange("(o r) c -> r o c", o=2),
                ysb.rearrange("r (o c) -> r o c", o=2),
            )

        # ---- pipelined attention over one q block ----
        def attention_qb(qb, hooks):
            cq = psum.tile([128, 2048], F32, name="cq", tag="cq", bufs=1)
            pend = []
            for i in range(NIT + LAG):
                for f in hooks.get(i, []):
                    f()
                if i < NIT:
                    kt, pair = i // 2, i % 2
                    tq = psum.tile([128, 1024], F32, name="tq", tag="psa", bufs=2)
                    for hh in range(2):
                        nc.tensor.matmul(
                            tq[:, hh * 512:(hh + 1) * 512],
                            kt_sb[pair][hh * 64:(hh + 1) * 64,
                                        kt * 128:(kt + 1) * 128],
                            qt[pair][hh * 64:(hh + 1) * 64,
                                     qb * 512:(qb + 1) * 512],
                            start=True, stop=True,
                        )
                    ex = work.tile([128, 1024], BF16, name="ex", tag="ex", bufs=4)
                    nc.scalar.activation(ex[:], tq[:], EXP)
                    pt = work.tile([128, 1024], BF16, name="pt", tag="pt", bufs=6)
                    for hh in range(2):
                        nc.vector.tensor_mul(
                            pt[:, hh * 512:(hh + 1) * 512],
                            ex[:, hh * 512:(hh + 1) * 512],
                            mk[kt][:, qb * 512:(qb + 1) * 512],
                        )
                    pend.append((kt, pair, pt))
                if i >= LAG:
                    kt, pair, pt = pend.pop(0)
                    for hh in range(2):
                        h = pair * 2 + hh
                        nc.tensor.matmul(
                            cq[0:65, h * 512:(h + 1) * 512],
                            v_sb[kt][:, h * 65:h * 65 + 65],
                            pt[:, hh * 512:(hh + 1) * 512],
                            start=(kt == 0), stop=(kt == NKT - 1),
                        )
            return cq

        prev_cq = None
        for qb in range(4):
            if qb == 0:
                hooks = {0: [lambda: quarter(0)],
                         8: [lambda: quarter(1)],
                         16: [lambda: quarter(2)],
                         24: [lambda: quarter(3)]}
            else:
                pq, pcq = qb - 1, prev_cq
                hooks = {
                    0: [lambda pq=pq, pcq=pcq: boundary_head(pq, pcq)],
                    12: [lambda pq=pq: boundary_cn(pq)],
                    14: [lambda pq=pq: op_chunk(pq, 0)],
                    17: [lambda pq=pq: op_chunk(pq, 1)],
                    20: [lambda pq=pq: op_chunk(pq, 2)],
                    23: [lambda pq=pq: op_chunk(pq, 3)],
                }
            prev_cq = attention_qb(qb, hooks)

        boundary_head(3, prev_cq)
        boundary_cn(3)
        for g in range(4):
            op_chunk(3, g)

    nc.compile()
    return nc


def _get_nc():
    if "nc" not in _CACHE:
        _CACHE["nc"] = _build_nc()
    return _CACHE["nc"]


def kernel(x, mask, w_qkv, b_qkv, w_o, b_o):
    x = np.asarray(x, dtype=np.float32)
    mask = np.asarray(mask)
    w_qkv = np.asarray(w_qkv, dtype=np.float32)
    b_qkv = np.asarray(b_qkv, dtype=np.float32)
    w_o = np.asarray(w_o, dtype=np.float32)
    b_o = np.asarray(b_o, dtype=np.float32)
    assert not b_qkv.any(), "kernel specialized for zero qkv bias"

    scale = np.float32(1.0 / np.sqrt(HD))
    maskT = np.ascontiguousarray(mask.reshape(S, S).T).astype(ml_dtypes.bfloat16)

    w3 = w_qkv.reshape(H, 3, HD, D)  # [head, (q,k,v), hd, D]
    in_maps = []
    for c in range(N_CORES):
        b = c // 4
        h0 = (c % 4) * HL
        heads = list(range(h0, h0 + HL))
        wq = w3[heads, 0].reshape(CH, D) * scale
        wk = w3[heads, 1].reshape(CH, D)
        wv = w3[heads, 2].reshape(CH, D)
        wo_cols = np.concatenate([w_o[:, h * HD:(h + 1) * HD] for h in heads], axis=1)
        in_maps.append({
            "xT": np.ascontiguousarray(x[b].T).astype(ml_dtypes.bfloat16),
            "maskT": maskT,
            "wqT": np.ascontiguousarray(wq.T).astype(ml_dtypes.bfloat16),
            "wkT": np.ascontiguousarray(wk.T).astype(ml_dtypes.bfloat16),
            "wvT": np.ascontiguousarray(wv.T).astype(ml_dtypes.bfloat16),
            "woT": np.ascontiguousarray(wo_cols.T).astype(ml_dtypes.bfloat16),
        })

    nc = _get_nc()
    trace = bool(int(os.environ.get("MHA_TRACE", "0")))
    res = run_bass_kernel_spmd(nc, in_maps, core_ids=list(range(N_CORES)),
                               trace=trace)
    _CACHE["last_results"] = res

    y = np.zeros((B, S, D), dtype=np.float32)
    for c in range(N_CORES):
        y[c // 4] += res.results[c]["yT"].T
    y += b_o
    return y


# revision 11
# speedup vs baseline: 1.2749x; 1.0377x over previous
"""Multi-head attention (B=2, S=2048, D=1024, H=16) on 8 TRN2 NeuronCores.

Sharding: (batch, head-group) SPMD. Core c handles batch b = c//4 and local
heads [4*(c%4), 4*(c%4)+4). Each core computes its 4 heads' attention plus the
partial o-projection (row-parallel over the head dimension); the host sums the
4 partial outputs per batch and adds b_o.

v3 schedule (from v1/v2 traces):
  - phase-1 projection quarters interleaved into qb0's attention stream
  - tensor queue ordered S(i) ... PV(i-2) so the PE never micro-idles
  - ACT runs Exp only; the softmax denominator reciprocal runs on DVE after
    a DRAM round-trip reshape to [128,16] (native ops only)
  - batched DMAs (one per weight tensor / x quarter / 4-mask group) to kill
    the 615ns-per-issue ramp serialization seen in the v2 trace
  - o_proj chunks hooked late into the next qb so they never block S matmuls
  - last qb runs pair-major so its first half-boundary overlaps compute
"""
import os
import sys

if "/opt/trn_rl_repo" not in sys.path:
    sys.path.insert(0, "/opt/trn_rl_repo")
os.environ.setdefault("JAX_PLATFORMS", "axon,cpu")

from contextlib import ExitStack

import ml_dtypes
import numpy as np

import concourse.bass as bass
import concourse.tile as tile
from concourse import bacc, library_config, mybir
from concourse.bass_utils import run_bass_kernel_spmd

F32 = mybir.dt.float32
BF16 = mybir.dt.bfloat16
EXP = mybir.ActivationFunctionType.Exp

B, S, D = 2, 2048, 1024
H, HD = 16, 64
HL = 4            # local heads per core
CH = HL * HD      # 256 local channels
N_CORES = 8
KC = D // 128     # 8 contraction chunks for the projections
NKT = S // 128    # 16 k tiles
NIT = NKT * 2     # 32 (ktile, pair) iterations per q block
LAG = 2           # PV trails S by this many iterations in the tensor queue

_CACHE = {}


def _build_nc():
    nc = bacc.Bacc("TRN2", target_bir_lowering=False)
    xT_d = nc.declare_dram_parameter("xT", [D, S], BF16, isOutput=False)
    mk_d = nc.declare_dram_parameter("maskT", [S, S], BF16, isOutput=False)
    wqT_d = nc.declare_dram_parameter("wqT", [D, CH], BF16, isOutput=False)
    wkT_d = nc.declare_dram_parameter("wkT", [D, CH], BF16, isOutput=False)
    wvT_d = nc.declare_dram_parameter("wvT", [D, CH], BF16, isOutput=False)
    woT_d = nc.declare_dram_parameter("woT", [CH, D], BF16, isOutput=False)
    yT_d = nc.declare_dram_parameter("yT", [D, S], F32, isOutput=True)
    # scratch for the denominator reshape dance, per (qb, pair-half) slot
    scr_d = nc.declare_dram_parameter("scr", [8, 2, 1024], F32, isOutput=True)

    with tile.TileContext(nc) as tc, ExitStack() as ctx:
        nc.gpsimd.load_library(library_config.attn)
        const = ctx.enter_context(tc.tile_pool(name="const", bufs=1))
        work = ctx.enter_context(tc.tile_pool(name="work", bufs=1))
        psum = ctx.enter_context(tc.tile_pool(name="psum", bufs=1, space="PSUM"))

        # ---- resident tensors (batched DMAs) ----
        mk4 = [const.tile([128, 4, S], BF16, name=f"mk{g}") for g in range(4)]
        wo_all = const.tile([64, HL, D], BF16, name="wo")
        nc.sync.dma_start(wo_all[:], woT_d.rearrange("(h p) c -> p h c", p=64))
        qt = [const.tile([128, S], BF16, name=f"qt{i}") for i in range(2)]
        kt_sb = [const.tile([128, S], BF16, name=f"kt{i}") for i in range(2)]
        v_sb = [const.tile([128, HL * 65], BF16, name=f"v{i}") for i in range(NKT)]
        for st in range(NKT):
            nc.gpsimd.memset(
                v_sb[st].rearrange("p (h c) -> p h c", h=HL)[:, :, 64:65], 1.0
            )
        w_all = {}
        for nm, dram in (("wq", wqT_d), ("wk", wkT_d), ("wv", wvT_d)):
            t = const.tile([128, KC, CH], BF16, name=nm)
            nc.sync.dma_start(t[:], dram.rearrange("(k p) c -> p k c", p=128))
            w_all[nm] = t

        def mask_ap(kt, qb):
            return mk4[kt // 4][:, kt % 4, qb * 512:(qb + 1) * 512]

        # ---- phase-1 quarter: KT/V/QT projections for seq quarter qh ----
        xts = {}

        def load_quarter(qh):
            xt = work.tile([128, KC, 512], BF16, name="xt", tag="xt", bufs=2)
            nc.sync.dma_start(
                xt[:],
                xT_d[:, qh * 512:(qh + 1) * 512].rearrange("(k p) s -> p k s", p=128),
            )
            nc.sync.dma_start(
                mk4[qh][:],
                mk_d[qh * 512:(qh + 1) * 512, :].rearrange("(j p) s -> p j s", p=128),
            )
            xts[qh] = xt

        def quarter(qh):
            if qh == 0:
                load_quarter(0)
            xt = xts.pop(qh)
            for wsb, dsts in ((w_all["wk"], kt_sb), (w_all["wq"], qt)):
                ps = psum.tile([128, 1024], F32, name="p1qk", tag="psa", bufs=2)
                for mt in range(2):
                    for k in range(KC):
                        nc.tensor.matmul(
                            ps[:, mt * 512:(mt + 1) * 512],
                            wsb[:, k, mt * 128:(mt + 1) * 128],
                            xt[:, k, :],
                            start=(k == 0), stop=(k == KC - 1),
                        )
                for mt in range(2):
                    nc.scalar.copy(
                        dsts[mt][:, qh * 512:(qh + 1) * 512],
                        ps[:, mt * 512:(mt + 1) * 512],
                    )
                if wsb is w_all["wk"]:
                    vp = psum.tile([128, 1024], F32, name="p1v", tag="psa", bufs=2)
                    for sl in range(4):
                        for k in range(KC):
                            nc.tensor.matmul(
                                vp[:, sl * 256:(sl + 1) * 256],
                                xt[:, k, sl * 128:(sl + 1) * 128],
                                w_all["wv"][:, k, :],
                                start=(k == 0), stop=(k == KC - 1),
                            )
                    for sl in range(4):
                        st = qh * 4 + sl
                        nc.scalar.copy(
                            v_sb[st].rearrange("p (h c) -> p h c", h=HL)[:, :, 0:64],
                            vp[:, sl * 256:(sl + 1) * 256].rearrange(
                                "p (h c) -> p h c", h=HL
                            ),
                        )
            if qh < 3:
                load_quarter(qh + 1)

        # ---- half-boundary: reciprocal of one pair's denominators ----
        # slot = qb*2 + pair; processes cq columns [pair*1024, pair*1024+1024)
        bnd = {}

        def dance(slot, cq, pair):
            c0 = pair * 1024
            cnr = work.tile([65, 1024], F32, name=f"cnr{pair}", tag=f"cnr{pair}",
                            bufs=2)
            nc.scalar.copy(cnr[:], cq[0:65, c0:c0 + 1024])
            nc.sync.dma_start(
                scr_d[slot, 0, :].rearrange("(a c) -> a c", a=1), cnr[64:65, :]
            )
            r128 = work.tile([128, 8], F32, name="r128", tag="r128", bufs=2)
            nc.sync.dma_start(
                r128[:], scr_d[slot, 0, :].rearrange("(p c) -> p c", p=128)
            )
            nc.vector.reciprocal(r128[:], r128[:])
            nc.sync.dma_start(
                scr_d[slot, 1, :].rearrange("(p c) -> p c", p=128), r128[:]
            )
            rb = work.tile([64, 1024], F32, name=f"rb{pair}", tag=f"rb{pair}",
                           bufs=2)
            scr1 = scr_d[slot, 1, :]
            nc.sync.dma_start(
                rb[:], bass.AP(scr1.tensor, scr1.offset, [(0, 64), (1, 1024)])
            )
            bnd[("cnr", slot)] = cnr
            bnd[("rb", slot)] = rb

        def cn_mul(slot, pair):
            cn = bnd.get(("cn", slot // 2))
            if cn is None:
                cn = work.tile([64, 2048], BF16, name="cn", tag="cn", bufs=2)
                bnd[("cn", slot // 2)] = cn
            c0 = pair * 1024
            nc.vector.tensor_mul(
                cn[:, c0:c0 + 1024],
                bnd[("cnr", slot)][0:64, :],
                bnd[("rb", slot)][:],
            )

        def op_chunk(qb, g):
            cn = bnd[("cn", qb)]
            op = psum.tile([128, 1024], F32, name="op", tag="psa", bufs=2)
            for j in range(2):
                ot = g * 2 + j
                for h in range(HL):
                    nc.tensor.matmul(
                        op[:, j * 512:(j + 1) * 512],
                        wo_all[:, h, ot * 128:(ot + 1) * 128],
                        cn[:, h * 512:(h + 1) * 512],
                        start=(h == 0), stop=(h == HL - 1),
                    )
            ysb = work.tile([128, 1024], F32, name="ysb", tag="ysb", bufs=2)
            nc.vector.tensor_copy(ysb[:], op[:])
            nc.sync.dma_start(
                yT_d[g * 256:(g + 1) * 256,
                     qb * 512:(qb + 1) * 512].rearrange("(o r) c -> r o c", o=2),
                ysb.rearrange("r (o c) -> r o c", o=2),
            )

        # ---- pipelined attention over one q block ----
        def attention_qb(qb, cq, hooks, pair_major=False):
            if pair_major:
                order = [(kt, p) for p in range(2) for kt in range(NKT)]
            else:
                order = [(kt, p) for kt in range(NKT) for p in range(2)]
            pend = []
            for i in range(NIT + LAG):
                for f in hooks.get(i, []):
                    f()
                if i < NIT:
                    kt, pair = order[i]
                    tq = psum.tile([128, 1024], F32, name="tq", tag="psa", bufs=2)
                    for hh in range(2):
                        nc.tensor.matmul(
                            tq[:, hh * 512:(hh + 1) * 512],
                            kt_sb[pair][hh * 64:(hh + 1) * 64,
                                        kt * 128:(kt + 1) * 128],
                            qt[pair][hh * 64:(hh + 1) * 64,
                                     qb * 512:(qb + 1) * 512],
                            start=True, stop=True,
                        )
                    ex = work.tile([128, 1024], BF16, name="ex", tag="ex", bufs=4)
                    nc.scalar.activation(ex[:], tq[:], EXP)
                    pt = work.tile([128, 1024], BF16, name="pt", tag="pt", bufs=6)
                    for hh in range(2):
                        nc.vector.tensor_mul(
                            pt[:, hh * 512:(hh + 1) * 512],
                            ex[:, hh * 512:(hh + 1) * 512],
                            mask_ap(kt, qb),
                        )
                    pend.append((kt, pair, pt))
                if i >= LAG:
                    kt, pair, pt = pend.pop(0)
                    for hh in range(2):
                        h = pair * 2 + hh
                        nc.tensor.matmul(
                            cq[0:65, h * 512:(h + 1) * 512],
                            v_sb[kt][:, h * 65:h * 65 + 65],
                            pt[:, hh * 512:(hh + 1) * 512],
                            start=(kt == 0), stop=(kt == NKT - 1),
                        )

        def boundary_hooks(pq, pcq):
            # full boundary for q-block pq, interleaved into the next block
            return {
                0: [lambda: dance(pq * 2, pcq, 0),
                    lambda: dance(pq * 2 + 1, pcq, 1)],
                6: [lambda: cn_mul(pq * 2, 0), lambda: cn_mul(pq * 2 + 1, 1)],
                10: [lambda: op_chunk(pq, 0)],
                14: [lambda: op_chunk(pq, 1)],
                18: [lambda: op_chunk(pq, 2)],
                22: [lambda: op_chunk(pq, 3)],
            }

        prev_cq = None
        for qb in range(4):
            cq = psum.tile([128, 2048], F32, name="cq", tag="cq", bufs=1)
            if qb == 0:
                hooks = {0: [lambda: quarter(0)],
                         8: [lambda: quarter(1)],
                         16: [lambda: quarter(2)],
                         24: [lambda: quarter(3)]}
            else:
                hooks = boundary_hooks(qb - 1, prev_cq)
            if qb == 3:
                # pair-major: pair 0 finishes at i=15 so its half-boundary
                # overlaps pair 1's compute
                hooks.setdefault(18, []).append(lambda cq=cq: dance(6, cq, 0))
                hooks.setdefault(24, []).append(lambda: cn_mul(6, 0))
            attention_qb(qb, cq, hooks, pair_major=(qb == 3))
            prev_cq = cq

        # tail: second half-boundary of qb3 + its o_proj
        dance(7, prev_cq, 1)
        cn_mul(7, 1)
        for g in range(4):
            op_chunk(3, g)

    nc.compile()
    return nc


def _get_nc():
    if "nc" not in _CACHE:
        _CACHE["nc"] = _build_nc()
    return _CACHE["nc"]


def kernel(x, mask, w_qkv, b_qkv, w_o, b_o):
    x = np.asarray(x, dtype=np.float32)
    mask = np.asarray(mask)
    w_qkv = np.asarray(w_qkv, dtype=np.float32)
    b_qkv = np.asarray(b_qkv, dtype=np.float32)
    w_o = np.asarray(w_o, dtype=np.float32)
    b_o = np.asarray(b_o, dtype=np.float32)
    assert not b_qkv.any(), "kernel specialized for zero qkv bias"

    scale = np.float32(1.0 / np.sqrt(HD))
    maskT = np.ascontiguousarray(mask.reshape(S, S).T).astype(ml_dtypes.bfloat16)

    w3 = w_qkv.reshape(H, 3, HD, D)  # [head, (q,k,v), hd, D]
    in_maps = []
    for c in range(N_CORES):
        b = c // 4
        h0 = (c % 4) * HL
        heads = list(range(h0, h0 + HL))
        wq = w3[heads, 0].reshape(CH, D) * scale
        wk = w3[heads, 1].reshape(CH, D)
        wv = w3[heads, 2].reshape(CH, D)
        wo_cols = np.concatenate([w_o[:, h * HD:(h + 1) * HD] for h in heads], axis=1)
        in_maps.append({
            "xT": np.ascontiguousarray(x[b].T).astype(ml_dtypes.bfloat16),
            "maskT": maskT,
            "wqT": np.ascontiguousarray(wq.T).astype(ml_dtypes.bfloat16),
            "wkT": np.ascontiguousarray(wk.T).astype(ml_dtypes.bfloat16),
            "wvT": np.ascontiguousarray(wv.T).astype(ml_dtypes.bfloat16),
            "woT": np.ascontiguousarray(wo_cols.T).astype(ml_dtypes.bfloat16),
        })

    nc = _get_nc()
    trace = bool(int(os.environ.get("MHA_TRACE", "0")))
    res = run_bass_kernel_spmd(nc, in_maps, core_ids=list(range(N_CORES)),
                               trace=trace)
    _CACHE["last_results"] = res

    y = np.zeros((B, S, D), dtype=np.float32)
    for c in range(N_CORES):
        y[c // 4] += res.results[c]["yT"].T
    y += b_o
    return y


# revision 14
# speedup vs baseline: 1.3035x; 1.0224x over previous
"""Multi-head attention (B=2, S=2048, D=1024, H=16) on 8 TRN2 NeuronCores.

Sharding: (batch, head-group) SPMD. Core c handles batch b = c//4 and local
heads [4*(c%4), 4*(c%4)+4). Each core computes its 4 heads' attention plus the
partial o-projection (row-parallel over the head dimension); the host sums the
4 partial outputs per batch and adds b_o.

v3 schedule (from v1/v2 traces):
  - phase-1 projection quarters interleaved into qb0's attention stream
  - tensor queue ordered S(i) ... PV(i-2) so the PE never micro-idles
  - ACT runs Exp only; the softmax denominator reciprocal runs on DVE after
    a DRAM round-trip reshape to [128,16] (native ops only)
  - batched DMAs (one per weight tensor / x quarter / 4-mask group) to kill
    the 615ns-per-issue ramp serialization seen in the v2 trace
  - o_proj chunks hooked late into the next qb so they never block S matmuls
  - last qb runs pair-major so its first half-boundary overlaps compute
"""
import os
import sys

if "/opt/trn_rl_repo" not in sys.path:
    sys.path.insert(0, "/opt/trn_rl_repo")
os.environ.setdefault("JAX_PLATFORMS", "axon,cpu")

from contextlib import ExitStack

import ml_dtypes
import numpy as np

import concourse.bass as bass
import concourse.tile as tile
from concourse import bacc, library_config, mybir
from concourse.bass_utils import run_bass_kernel_spmd

F32 = mybir.dt.float32
BF16 = mybir.dt.bfloat16
EXP = mybir.ActivationFunctionType.Exp

B, S, D = 2, 2048, 1024
H, HD = 16, 64
HL = 4            # local heads per core
CH = HL * HD      # 256 local channels
N_CORES = 8
KC = D // 128     # 8 contraction chunks for the projections
NKT = S // 128    # 16 k tiles
NIT = NKT * 2     # 32 (ktile, pair) iterations per q block
LAG = 2           # PV trails S by this many iterations in the tensor queue

_CACHE = {}


def _build_nc():
    nc = bacc.Bacc("TRN2", target_bir_lowering=False)
    xT_d = nc.declare_dram_parameter("xT", [D, S], BF16, isOutput=False)
    mk_d = nc.declare_dram_parameter("maskT", [S, S], BF16, isOutput=False)
    wqT_d = nc.declare_dram_parameter("wqT", [D, CH], BF16, isOutput=False)
    wkT_d = nc.declare_dram_parameter("wkT", [D, CH], BF16, isOutput=False)
    wvT_d = nc.declare_dram_parameter("wvT", [D, CH], BF16, isOutput=False)
    woT_d = nc.declare_dram_parameter("woT", [CH, D], BF16, isOutput=False)
    yT_d = nc.declare_dram_parameter("yT", [D, S], F32, isOutput=True)
    # scratch for the denominator reshape dance, per (qb, pair-half) slot
    scr_d = nc.declare_dram_parameter("scr", [8, 2, 1024], F32, isOutput=True)

    with tile.TileContext(nc) as tc, ExitStack() as ctx:
        nc.gpsimd.load_library(library_config.attn)
        const = ctx.enter_context(tc.tile_pool(name="const", bufs=1))
        work = ctx.enter_context(tc.tile_pool(name="work", bufs=1))
        psum = ctx.enter_context(tc.tile_pool(name="psum", bufs=1, space="PSUM"))

        # ---- resident tensors (batched DMAs) ----
        mk4 = [const.tile([128, 4, S], BF16, name=f"mk{g}") for g in range(4)]
        wo_all = const.tile([64, HL, D], BF16, name="wo")
        nc.sync.dma_start(wo_all[:], woT_d.rearrange("(h p) c -> p h c", p=64))
        qt = [const.tile([128, S], BF16, name=f"qt{i}") for i in range(2)]
        kt_sb = [const.tile([128, S], BF16, name=f"kt{i}") for i in range(2)]
        v_sb = [const.tile([128, HL * 65], BF16, name=f"v{i}") for i in range(NKT)]
        for st in range(NKT):
            nc.gpsimd.memset(
                v_sb[st].rearrange("p (h c) -> p h c", h=HL)[:, :, 64:65], 1.0
            )
        w_all = {}
        for nm, dram in (("wq", wqT_d), ("wk", wkT_d), ("wv", wvT_d)):
            t = const.tile([128, KC, CH], BF16, name=nm)
            nc.sync.dma_start(t[:], dram.rearrange("(k p) c -> p k c", p=128))
            w_all[nm] = t

        def mask_ap(kt, qb):
            return mk4[kt // 4][:, kt % 4, qb * 512:(qb + 1) * 512]

        # ---- phase-1 quarter: KT/V/QT projections for seq quarter qh ----
        xts = {}

        def load_quarter(qh):
            xt = work.tile([128, KC, 512], BF16, name="xt", tag="xt", bufs=2)
            nc.sync.dma_start(
                xt[:],
                xT_d[:, qh * 512:(qh + 1) * 512].rearrange("(k p) s -> p k s", p=128),
            )
            nc.sync.dma_start(
                mk4[qh][:],
                mk_d[qh * 512:(qh + 1) * 512, :].rearrange("(j p) s -> p j s", p=128),
            )
            xts[qh] = xt

        def quarter_qk(qh, wsb, dsts):
            xt = xts[qh]
            ps = psum.tile([128, 1024], F32, name="p1qk", tag="psa", bufs=2)
            for mt in range(2):
                for k in range(KC):
                    nc.tensor.matmul(
                        ps[:, mt * 512:(mt + 1) * 512],
                        wsb[:, k, mt * 128:(mt + 1) * 128],
                        xt[:, k, :],
                        start=(k == 0), stop=(k == KC - 1),
                    )
            for mt in range(2):
                nc.scalar.copy(
                    dsts[mt][:, qh * 512:(qh + 1) * 512],
                    ps[:, mt * 512:(mt + 1) * 512],
                )

        def quarter_v(qh):
            xt = xts[qh]
            vp = psum.tile([128, 1024], F32, name="p1v", tag="psa", bufs=2)
            for sl in range(4):
                for k in range(KC):
                    nc.tensor.matmul(
                        vp[:, sl * 256:(sl + 1) * 256],
                        xt[:, k, sl * 128:(sl + 1) * 128],
                        w_all["wv"][:, k, :],
                        start=(k == 0), stop=(k == KC - 1),
                    )
            for sl in range(4):
                st = qh * 4 + sl
                nc.scalar.copy(
                    v_sb[st].rearrange("p (h c) -> p h c", h=HL)[:, :, 0:64],
                    vp[:, sl * 256:(sl + 1) * 256].rearrange(
                        "p (h c) -> p h c", h=HL
                    ),
                )

        def quarter_tail(qh):
            xts.pop(qh)
            if qh < 3:
                load_quarter(qh + 1)

        # ---- half-boundary: reciprocal of one pair's denominators ----
        # slot = qb*2 + pair; processes cq columns [pair*1024, pair*1024+1024)
        bnd = {}

        def dance(slot, cq, pair):
            c0 = pair * 1024
            cnr = work.tile([65, 1024], F32, name=f"cnr{pair}", tag=f"cnr{pair}",
                            bufs=2)
            nc.scalar.copy(cnr[:], cq[0:65, c0:c0 + 1024])
            nc.sync.dma_start(
                scr_d[slot, 0, :].rearrange("(a c) -> a c", a=1), cnr[64:65, :]
            )
            r128 = work.tile([128, 8], F32, name="r128", tag="r128", bufs=2)
            nc.sync.dma_start(
                r128[:], scr_d[slot, 0, :].rearrange("(p c) -> p c", p=128)
            )
            nc.vector.reciprocal(r128[:], r128[:])
            nc.sync.dma_start(
                scr_d[slot, 1, :].rearrange("(p c) -> p c", p=128), r128[:]
            )
            rb = work.tile([64, 1024], F32, name=f"rb{pair}", tag=f"rb{pair}",
                           bufs=2)
            scr1 = scr_d[slot, 1, :]
            nc.sync.dma_start(
                rb[:], bass.AP(scr1.tensor, scr1.offset, [(0, 64), (1, 1024)])
            )
            bnd[("cnr", slot)] = cnr
            bnd[("rb", slot)] = rb

        def cn_mul(slot, pair):
            cn = bnd.get(("cn", slot // 2))
            if cn is None:
                cn = work.tile([64, 2048], BF16, name="cn", tag="cn", bufs=2)
                bnd[("cn", slot // 2)] = cn
            c0 = pair * 1024
            nc.vector.tensor_mul(
                cn[:, c0:c0 + 1024],
                bnd[("cnr", slot)][0:64, :],
                bnd[("rb", slot)][:],
            )

        op_live = {}

        def op_chunk(qb, g, half=None):
            # half=None: both head pairs; half=0/1: only that pair's
            # contraction (accumulated across two calls)
            cn = bnd[("cn", qb)]
            if half in (None, 0):
                op = psum.tile([128, 1024], F32, name="op", tag="psa", bufs=2)
                op_live[(qb, g)] = op
            else:
                op = op_live[(qb, g)]
            hs = range(HL) if half is None else range(half * 2, half * 2 + 2)
            for j in range(2):
                ot = g * 2 + j
                for h in hs:
                    nc.tensor.matmul(
                        op[:, j * 512:(j + 1) * 512],
                        wo_all[:, h, ot * 128:(ot + 1) * 128],
                        cn[:, h * 512:(h + 1) * 512],
                        start=(h == hs[0] if half != 1 else False),
                        stop=(h == hs[-1] if half != 0 else False),
                    )
            if half == 0:
                return
            ysb = work.tile([128, 1024], F32, name="ysb", tag="ysb", bufs=2)
            nc.vector.tensor_copy(ysb[:], op[:])
            nc.sync.dma_start(
                yT_d[g * 256:(g + 1) * 256,
                     qb * 512:(qb + 1) * 512].rearrange("(o r) c -> r o c", o=2),
                ysb.rearrange("r (o c) -> r o c", o=2),
            )

        # ---- pipelined attention over one q block ----
        def attention_qb(qb, cq, hooks, pair_major=False):
            if pair_major:
                order = [(kt, p) for p in range(2) for kt in range(NKT)]
            else:
                order = [(kt, p) for kt in range(NKT) for p in range(2)]
            pend = []
            for i in range(NIT + LAG):
                for f in hooks.get(i, []):
                    f()
                if i < NIT:
                    kt, pair = order[i]
                    tq = psum.tile([128, 1024], F32, name="tq", tag="psa", bufs=2)
                    for hh in range(2):
                        nc.tensor.matmul(
                            tq[:, hh * 512:(hh + 1) * 512],
                            kt_sb[pair][hh * 64:(hh + 1) * 64,
                                        kt * 128:(kt + 1) * 128],
                            qt[pair][hh * 64:(hh + 1) * 64,
                                     qb * 512:(qb + 1) * 512],
                            start=True, stop=True,
                        )
                    ex = work.tile([128, 1024], BF16, name="ex", tag="ex", bufs=4)
                    nc.scalar.activation(ex[:], tq[:], EXP)
                    pt = work.tile([128, 1024], BF16, name="pt", tag="pt", bufs=6)
                    for hh in range(2):
                        nc.vector.tensor_mul(
                            pt[:, hh * 512:(hh + 1) * 512],
                            ex[:, hh * 512:(hh + 1) * 512],
                            mask_ap(kt, qb),
                        )
                    pend.append((kt, pair, pt))
                if i >= LAG:
                    kt, pair, pt = pend.pop(0)
                    for hh in range(2):
                        h = pair * 2 + hh
                        nc.tensor.matmul(
                            cq[0:65, h * 512:(h + 1) * 512],
                            v_sb[kt][:, h * 65:h * 65 + 65],
                            pt[:, hh * 512:(hh + 1) * 512],
                            start=(kt == 0), stop=(kt == NKT - 1),
                        )

        def boundary_hooks(pq, pcq):
            # full boundary for q-block pq, interleaved into the next block;
            # cn_mul sits late enough that the rb DMA dance (~7.5us) is done
            # before it enters the DVE FIFO
            return {
                0: [lambda: dance(pq * 2, pcq, 0),
                    lambda: dance(pq * 2 + 1, pcq, 1)],
                10: [lambda: cn_mul(pq * 2, 0), lambda: cn_mul(pq * 2 + 1, 1)],
                14: [lambda: op_chunk(pq, 0)],
                18: [lambda: op_chunk(pq, 1)],
                22: [lambda: op_chunk(pq, 2)],
                26: [lambda: op_chunk(pq, 3)],
            }

        prev_cq = None
        for qb in range(4):
            cq = psum.tile([128, 2048], F32, name="cq", tag="cq", bufs=1)
            if qb == 0:
                hooks = {
                    0: [lambda: load_quarter(0),
                        lambda: quarter_qk(0, w_all["wk"], kt_sb),
                        lambda: quarter_v(0),
                        lambda: quarter_qk(0, w_all["wq"], qt),
                        lambda: quarter_tail(0)],
                }
                for j in (1, 2, 3):
                    hooks[8 * j] = [lambda j=j: quarter_qk(j, w_all["wk"], kt_sb)]
                    hooks[8 * j + 2] = [lambda j=j: quarter_v(j)]
                    hooks[8 * j + 4] = [lambda j=j: quarter_qk(j, w_all["wq"], qt),
                                        lambda j=j: quarter_tail(j)]
            else:
                hooks = boundary_hooks(qb - 1, prev_cq)
            if qb == 3:
                # pair-major: pair 0 finishes at i=15 so its half-boundary
                # overlaps pair 1's compute
                hooks.setdefault(18, []).append(lambda cq=cq: dance(6, cq, 0))
                hooks.setdefault(28, []).append(lambda: cn_mul(6, 0))
            attention_qb(qb, cq, hooks, pair_major=(qb == 3))
            prev_cq = cq

        # tail: qb3's second half-boundary; pair-0 o_proj halves run during
        # the dance to keep the PE warm, pair-1 halves after cn
        dance(7, prev_cq, 1)
        op_chunk(3, 0, half=0)
        op_chunk(3, 1, half=0)
        cn_mul(7, 1)
        op_chunk(3, 0, half=1)
        op_chunk(3, 1, half=1)
        op_chunk(3, 2)
        op_chunk(3, 3)

    nc.compile()
    return nc


def _get_nc():
    if "nc" not in _CACHE:
        _CACHE["nc"] = _build_nc()
    return _CACHE["nc"]


def kernel(x, mask, w_qkv, b_qkv, w_o, b_o):
    x = np.asarray(x, dtype=np.float32)
    mask = np.asarray(mask)
    w_qkv = np.asarray(w_qkv, dtype=np.float32)
    b_qkv = np.asarray(b_qkv, dtype=np.float32)
    w_o = np.asarray(w_o, dtype=np.float32)
    b_o = np.asarray(b_o, dtype=np.float32)
    assert not b_qkv.any(), "kernel specialized for zero qkv bias"

    scale = np.float32(1.0 / np.sqrt(HD))
    maskT = np.ascontiguousarray(mask.reshape(S, S).T).astype(ml_dtypes.bfloat16)

    w3 = w_qkv.reshape(H, 3, HD, D)  # [head, (q,k,v), hd, D]
    in_maps = []
    for c in range(N_CORES):
        b = c // 4
        h0 = (c % 4) * HL
        heads = list(range(h0, h0 + HL))
        wq = w3[heads, 0].reshape(CH, D) * scale
        wk = w3[heads, 1].reshape(CH, D)
        wv = w3[heads, 2].reshape(CH, D)
        wo_cols = np.concatenate([w_o[:, h * HD:(h + 1) * HD] for h in heads], axis=1)
        in_maps.append({
            "xT": np.ascontiguousarray(x[b].T).astype(ml_dtypes.bfloat16),
            "maskT": maskT,
            "wqT": np.ascontiguousarray(wq.T).astype(ml_dtypes.bfloat16),
            "wkT": np.ascontiguousarray(wk.T).astype(ml_dtypes.bfloat16),
            "wvT": np.ascontiguousarray(wv.T).astype(ml_dtypes.bfloat16),
            "woT": np.ascontiguousarray(wo_cols.T).astype(ml_dtypes.bfloat16),
        })

    nc = _get_nc()
    trace = bool(int(os.environ.get("MHA_TRACE", "0")))
    res = run_bass_kernel_spmd(nc, in_maps, core_ids=list(range(N_CORES)),
                               trace=trace)
    _CACHE["last_results"] = res

    y = np.zeros((B, S, D), dtype=np.float32)
    for c in range(N_CORES):
        y[c // 4] += res.results[c]["yT"].T
    y += b_o
    return y
